# revision 9
# baseline (speedup 1.0000x reference)
"""Trainium2 Bass kernel for a ragged-sequence LSTM decoder (LSTMCell + FC head).

Contract: kernel(**inputs) takes the FULL unsharded inputs and returns the FULL
output [B, T, V] in length-sorted batch order (matching the reference).

Strategy (8 NeuronCores, SPMD single NEFF):
  - host: stable-sort batch by length desc; permute gates to [f,o,i,g]; transpose
    weights/activations; pad vocab 10000 -> 10240 (1280 per core).
  - phase A: input projection x @ W_ih^T, T-sharded (16 steps/core), AllGather.
    Every core also computes the t<16 slab locally so stepping starts before the
    AllGather lands.
  - phase B: 128 sequential LSTM steps, replicated on every core (batch 64 is
    too small to shard without per-step collectives). The 4096-wide gate matmul
    is split across the two 64-column halves of the PE array via tile_position
    col-tiling. Ragged lengths: only the first n_t sorted rows are updated.
  - FC head: vocab-sharded (1280/core), batched over 2 steps (M=128); b_fc is
    added on the host (it is zeros in this problem); output DMA'd to DRAM.
"""

import numpy as np
import ml_dtypes

try:
    import concourse.bass as bass  # noqa: F401
except ImportError:
    import sys
    sys.path.insert(0, "/opt/trn_rl_repo")
import concourse.bass as bass
import concourse.bacc as bacc
import concourse.mybir as mybir
import concourse.tile as tile
from concourse.bass_utils import run_bass_kernel_spmd

N_CORES = 8
B, T, E, D = 64, 128, 1024, 1024
V = 10000
VP = 10240            # padded vocab
VC = VP // N_CORES    # 1280 per core
G4 = 4 * D            # 4096 gates
TS = T // N_CORES     # 16 timesteps per core for the input projection

F32 = mybir.dt.float32
BF16 = mybir.dt.bfloat16
BF_NP = ml_dtypes.bfloat16
SIG = mybir.ActivationFunctionType.Sigmoid
TANH = mybir.ActivationFunctionType.Tanh

_CACHE = {}


def _build(n_t):
    """Build + compile the SPMD Bass program for a given active-rows schedule."""
    nc = bacc.Bacc("TRN2", target_bir_lowering=False, debug=False,
                   num_devices=N_CORES)

    # --- per-core DRAM I/O ---------------------------------------------------
    encT_own_d = nc.dram_tensor("encT_own", [E, TS, B], BF16, kind="ExternalInput")
    encT_first_d = nc.dram_tensor("encT_first", [E, TS, B], BF16, kind="ExternalInput")
    wih_d = nc.dram_tensor("wihT", [E, G4], BF16, kind="ExternalInput")
    whh_d = nc.dram_tensor("whhT", [D, G4], BF16, kind="ExternalInput")
    wfc_d = nc.dram_tensor("wfcT", [D, VC], BF16, kind="ExternalInput")
    bias_d = nc.dram_tensor("bias_rep", [128, G4], F32, kind="ExternalInput")
    ident_d = nc.dram_tensor("ident", [B, B], BF16, kind="ExternalInput")
    out_d = nc.dram_tensor("preds", [B, T, VC], F32, kind="ExternalOutput")

    with tile.TileContext(nc) as tc:
        with (
            tc.tile_pool(name="dram", bufs=1, space="DRAM") as dpool,
            tc.tile_pool(name="psum", bufs=1, space="PSUM") as psum,
            tc.tile_pool(name="persist", bufs=1) as pp,
        ):
            xp_own = dpool.tile([TS * B, G4], F32, tag="xp_own")
            xp_first = dpool.tile([TS * B, G4], F32, tag="xp_first")
            xp_ag = dpool.tile([T * B, G4], F32, tag="xp_ag", addr_space="Shared")

            # persistent state + phase-B weights
            whh = pp.tile([128, 8, G4], BF16, tag="whh")
            wfc = pp.tile([128, 8, VC], BF16, tag="wfc")
            ident = pp.tile([B, B], BF16, tag="ident")
            cst = pp.tile([B, D], F32, tag="cst")
            hT = pp.tile([128, 8, B], BF16, tag="hT")
            hT2 = pp.tile([128, 8, 2 * B], BF16, tag="hT2")

            nc.sync.dma_start(out=whh[:], in_=whh_d.ap().rearrange("(k p) n -> p k n", p=128))
            nc.sync.dma_start(out=wfc[:], in_=wfc_d.ap().rearrange("(k p) n -> p k n", p=128))
            nc.sync.dma_start(out=ident[:], in_=ident_d[:])
            nc.vector.memzero(cst[:])
            nc.vector.memzero(hT[:])

            # one shared PSUM accumulator tile (4 banks) for xproj + recurrence
            ps = psum.tile([128, 2048], F32, tag="ps", name="ps", bufs=1)

            # ---------------- phase A: input projection + AllGather ----------
            with tc.tile_pool(name="pha", bufs=1) as pa:
                bias = pa.tile([128, G4], F32, tag="bias")
                nc.sync.dma_start(out=bias[:], in_=bias_d[:])
                wih_view = wih_d.ap().rearrange("(k p) n -> p k n", p=128)
                encTs = []
                for sh, src_d in enumerate([(encT_first_d), (encT_own_d)]):
                    encT = pa.tile([128, 8, TS * B], BF16, tag=f"encT{sh}",
                                   name=f"encT{sh}")
                    nc.sync.dma_start(
                        out=encT[:],
                        in_=src_d.ap().rearrange("(k p) t b -> p k (t b)", p=128))
                    encTs.append(encT)
                for n in range(8):
                    wih_n = pa.tile([128, 8, 512], BF16, tag="wih_n",
                                    name="wih_n", bufs=2)
                    nc.sync.dma_start(out=wih_n[:],
                                      in_=wih_view[:, :, n * 512:(n + 1) * 512])
                    for sh, dst in enumerate([xp_first, xp_own]):
                        for m in range(8):
                            w4 = (n * 16 + sh * 8 + m) % 4
                            sl = ps[:, w4 * 512:(w4 + 1) * 512]
                            for k in range(8):
                                nc.tensor.matmul(
                                    sl, encTs[sh][:, k, m * 128:(m + 1) * 128],
                                    wih_n[:, k, :],
                                    start=(k == 0), stop=(k == 7))
                            xp_sb = pa.tile([128, 512], F32, tag="xp_sb",
                                            name="xp_sb", bufs=3)
                            nc.vector.tensor_add(
                                xp_sb[:], sl, bias[:, n * 512:(n + 1) * 512])
                            nc.sync.dma_start(
                                out=dst[m * 128:(m + 1) * 128, n * 512:(n + 1) * 512],
                                in_=xp_sb[:])
                nc.gpsimd.collective_compute(
                    "AllGather", mybir.AluOpType.bypass,
                    replica_groups=[list(range(N_CORES))],
                    ins=[xp_own.opt()], outs=[xp_ag.opt()])

            # ---------------- phase B: recurrence + FC head -------------------
            with tc.tile_pool(name="phb", bufs=1) as pb:
                for t in range(T):
                    n = int(n_t[t])
                    half = t % 2
                    if n > 0:
                        xp_src = xp_first if t < TS else xp_ag
                        xpt = pb.tile([128, 2048], F32, tag="xpt", name="xpt", bufs=2)
                        nc.sync.dma_start(
                            out=xpt[0:64, :], in_=xp_src[B * t:B * (t + 1), 0:2048])
                        nc.sync.dma_start(
                            out=xpt[64:128, :], in_=xp_src[B * t:B * (t + 1), 2048:4096])
                        for w in range(4):
                            sl = ps[:, w * 512:(w + 1) * 512]
                            for g in range(2):
                                for k in range(8):
                                    nc.tensor.matmul(
                                        sl[g * 64:(g + 1) * 64, :],
                                        hT[:, k, :],
                                        whh[:, k, g * 2048 + w * 512:
                                            g * 2048 + (w + 1) * 512],
                                        start=(k == 0), stop=(k == 7),
                                        tile_position=(0, g * 64))
                            nc.vector.tensor_add(
                                sl, sl, xpt[:, w * 512:(w + 1) * 512])
                        sig = pb.tile([128, 2048], F32, tag="sig", name="sig", bufs=1)
                        # cols 0:1024 = {f (top), i (bottom)} both sigmoid
                        nc.scalar.activation(sig[:, 0:1024], ps[:, 0:1024], SIG)
                        # cols 1024:2048 = {o (top) sigmoid, g (bottom) tanh}
                        nc.scalar.activation(sig[0:64, 1024:2048],
                                             ps[0:64, 1024:2048], SIG)
                        nc.scalar.activation(sig[64:128, 1024:2048],
                                             ps[64:128, 1024:2048], TANH)
                        fc_t = pb.tile([B, D], F32, tag="fc_t", name="fc_t", bufs=1)
                        ig = pb.tile([128, D], F32, tag="ig", name="ig", bufs=1)
                        igt = pb.tile([B, D], F32, tag="igt", name="igt", bufs=1)
                        th = pb.tile([B, D], F32, tag="th", name="th", bufs=1)
                        hbf = pb.tile([B, D], BF16, tag="hbf", name="hbf", bufs=1)
                        nc.vector.tensor_mul(fc_t[0:n, :], sig[0:n, 0:1024],
                                             cst[0:n, :])
                        nc.vector.tensor_mul(ig[64:64 + n, :], sig[64:64 + n, 0:1024],
                                             sig[64:64 + n, 1024:2048])
                        nc.sync.dma_start(out=igt[0:n, :], in_=ig[64:64 + n, :])
                        nc.vector.tensor_add(cst[0:n, :], fc_t[0:n, :], igt[0:n, :])
                        nc.scalar.activation(th[0:n, :], cst[0:n, :], TANH)
                        nc.vector.tensor_mul(hbf[0:n, :], sig[0:n, 1024:2048],
                                             th[0:n, :])
                        ps_t = psum.tile([128, 8, B], BF16, tag="ps_t",
                                         name="ps_t", bufs=1)
                        for k in range(8):
                            nc.tensor.transpose(
                                ps_t[:, k, 0:n], hbf[0:n, k * 128:(k + 1) * 128],
                                ident[0:n, 0:n])
                        nc.scalar.copy(hT[:, :, 0:n], ps_t[:, :, 0:n])
                    nc.vector.tensor_copy(hT2[:, :, half * B:(half + 1) * B], hT[:])
                    if half == 1:
                        fps = psum.tile([128, VC], F32, tag="fps", name="fps", bufs=1)
                        for nn in range(3):
                            c0, c1 = nn * 512, min((nn + 1) * 512, VC)
                            for k in range(8):
                                nc.tensor.matmul(
                                    fps[:, c0:c1], hT2[:, k, :], wfc[:, k, c0:c1],
                                    start=(k == 0), stop=(k == 7))
                        prd = pb.tile([128, VC], F32, tag="prd", name="prd", bufs=2)
                        nc.scalar.copy(prd[:], fps[:])
                        nc.sync.dma_start(out=out_d.ap()[:, t - 1, :],
                                          in_=prd[0:64, :])
                        nc.sync.dma_start(out=out_d.ap()[:, t, :],
                                          in_=prd[64:128, :])

    nc.compile()
    return nc


def _host_prep(inputs):
    enc = np.asarray(inputs["encoder_out"], np.float32)
    lens = np.asarray(inputs["caption_lengths"]).reshape(-1).astype(np.int64)
    W_ih = np.asarray(inputs["W_ih"], np.float32)
    b_ih = np.asarray(inputs["b_ih"], np.float32)
    W_hh = np.asarray(inputs["W_hh"], np.float32)
    b_hh = np.asarray(inputs["b_hh"], np.float32)
    W_fc = np.asarray(inputs["W_fc"], np.float32)
    b_fc = np.asarray(inputs["b_fc"], np.float32)

    sort_ind = np.argsort(-lens, kind="stable")
    dec_len = lens[sort_ind] - 1
    n_t = np.array([(dec_len > t).sum() for t in range(T)], np.int64)
    enc_s = enc[sort_ind]

    # gate permutation i,f,g,o -> f,o,i,g
    perm = np.concatenate([np.arange(D, 2 * D), np.arange(3 * D, 4 * D),
                           np.arange(0, D), np.arange(2 * D, 3 * D)])
    wihT = np.ascontiguousarray(W_ih[perm].T).astype(BF_NP)          # [E, 4D]
    whhT = np.ascontiguousarray(W_hh[perm].T).astype(BF_NP)          # [D, 4D]
    bias = (b_ih + b_hh)[perm].astype(np.float32)
    bias_rep = np.ascontiguousarray(np.broadcast_to(bias, (128, G4)))

    wfc_pad = np.zeros((VP, D), np.float32)
    wfc_pad[:V] = W_fc
    bfc_pad = np.zeros((VP,), np.float32)
    bfc_pad[:V] = b_fc

    encT = np.ascontiguousarray(enc_s.transpose(2, 1, 0)).astype(BF_NP)  # [E,T,B]
    ident = np.eye(B, dtype=BF_NP)

    in_maps = []
    for c in range(N_CORES):
        wfcT_c = np.ascontiguousarray(
            wfc_pad[c * VC:(c + 1) * VC].T).astype(BF_NP)            # [D, VC]
        in_maps.append({
            "encT_own": np.ascontiguousarray(encT[:, c * TS:(c + 1) * TS, :]),
            "encT_first": np.ascontiguousarray(encT[:, 0:TS, :]),
            "wihT": wihT,
            "whhT": whhT,
            "wfcT": wfcT_c,
            "bias_rep": bias_rep,
            "ident": ident,
        })
    return in_maps, n_t, bfc_pad


def kernel(**inputs):
    in_maps, n_t, bfc_pad = _host_prep(inputs)
    key = tuple(n_t.tolist())
    if key not in _CACHE:
        _CACHE[key] = _build(n_t)
    nc = _CACHE[key]
    res = run_bass_kernel_spmd(nc, in_maps, core_ids=list(range(N_CORES)))
    out = np.concatenate([res.results[c]["preds"] for c in range(N_CORES)],
                         axis=2)[:, :, :V]
    out = out.astype(np.float32)
    if bfc_pad.any():
        out += bfc_pad[:V]
    return np.ascontiguousarray(out)


# revision 11
# speedup vs baseline: 26.5516x; 26.5516x over previous
"""Trainium2 Bass kernel for a ragged-sequence LSTM decoder (LSTMCell + FC head).

Contract: kernel(**inputs) takes the FULL unsharded inputs and returns the FULL
output [B, T, V] in length-sorted batch order (matching the reference).

Strategy (8 NeuronCores, SPMD single NEFF):
  - host: stable-sort batch by length desc; permute gates to [f,o,i,g]; transpose
    weights/activations; pad vocab 10000 -> 10240 (1280 per core).
  - phase A: input projection x @ W_ih^T, T-sharded (16 steps/core), AllGather.
    Every core also computes the t<16 slab locally so stepping starts before the
    AllGather lands.
  - phase B: 128 sequential LSTM steps, replicated on every core (batch 64 is
    too small to shard without per-step collectives). The 4096-wide gate matmul
    is split across the two 64-column halves of the PE array via tile_position
    col-tiling. Ragged lengths: only the first n_t sorted rows are updated.
  - FC head: vocab-sharded (1280/core), batched over 2 steps (M=128); b_fc is
    added on the host (it is zeros in this problem); output DMA'd to DRAM.
"""

import numpy as np
import ml_dtypes

try:
    import concourse.bass as bass  # noqa: F401
except ImportError:
    import sys
    sys.path.insert(0, "/opt/trn_rl_repo")
import concourse.bass as bass
import concourse.bacc as bacc
import concourse.mybir as mybir
import concourse.tile as tile
from concourse.bass_utils import run_bass_kernel_spmd

N_CORES = 8
B, T, E, D = 64, 128, 1024, 1024
V = 10000
VP = 10240            # padded vocab
VC = VP // N_CORES    # 1280 per core
G4 = 4 * D            # 4096 gates
TS = T // N_CORES     # 16 timesteps per core for the input projection

F32 = mybir.dt.float32
BF16 = mybir.dt.bfloat16
BF_NP = ml_dtypes.bfloat16
SIG = mybir.ActivationFunctionType.Sigmoid
TANH = mybir.ActivationFunctionType.Tanh

_CACHE = {}


def _build(n_t):
    """Build + compile the SPMD Bass program for a given active-rows schedule."""
    nc = bacc.Bacc("TRN2", target_bir_lowering=False, debug=False,
                   num_devices=N_CORES)

    # --- per-core DRAM I/O ---------------------------------------------------
    encT_own_d = nc.dram_tensor("encT_own", [E, TS, B], BF16, kind="ExternalInput")
    encT_first_d = nc.dram_tensor("encT_first", [E, TS, B], BF16, kind="ExternalInput")
    wih_d = nc.dram_tensor("wihT", [E, G4], BF16, kind="ExternalInput")
    whh_d = nc.dram_tensor("whhT", [D, G4], BF16, kind="ExternalInput")
    wfc_d = nc.dram_tensor("wfcT", [D, VC], BF16, kind="ExternalInput")
    bias_d = nc.dram_tensor("bias_rep", [128, G4], F32, kind="ExternalInput")
    ident_d = nc.dram_tensor("ident", [B, B], BF16, kind="ExternalInput")
    out_d = nc.dram_tensor("preds", [B, T, VC], F32, kind="ExternalOutput")

    with tile.TileContext(nc) as tc:
        with (
            tc.tile_pool(name="dram", bufs=1, space="DRAM") as dpool,
            tc.tile_pool(name="psum", bufs=1, space="PSUM") as psum,
            tc.tile_pool(name="persist", bufs=1) as pp,
        ):
            xp_own = dpool.tile([TS * B, G4], F32, tag="xp_own")
            xp_first = dpool.tile([TS * B, G4], F32, tag="xp_first")
            xp_ag = dpool.tile([T * B, G4], F32, tag="xp_ag", addr_space="Shared")

            # persistent state + phase-B weights
            whh = pp.tile([128, 8, G4], BF16, tag="whh")
            wfc = pp.tile([128, 8, VC], BF16, tag="wfc")
            ident = pp.tile([B, B], BF16, tag="ident")
            cst = pp.tile([B, D], F32, tag="cst")
            hT = pp.tile([128, 8, B], BF16, tag="hT")
            hT2 = pp.tile([128, 8, 2 * B], BF16, tag="hT2")

            nc.sync.dma_start(out=whh[:], in_=whh_d.ap().rearrange("(k p) n -> p k n", p=128))
            nc.sync.dma_start(out=wfc[:], in_=wfc_d.ap().rearrange("(k p) n -> p k n", p=128))
            nc.sync.dma_start(out=ident[:], in_=ident_d[:])
            nc.vector.memzero(cst[:])
            nc.vector.memzero(hT[:])

            # one shared PSUM accumulator tile (4 banks) for xproj + recurrence
            ps = psum.tile([128, 2048], F32, tag="ps", name="ps", bufs=1)

            # ---------------- phase A: input projection + AllGather ----------
            with tc.tile_pool(name="pha", bufs=1) as pa:
                bias = pa.tile([128, G4], F32, tag="bias")
                nc.sync.dma_start(out=bias[:], in_=bias_d[:])
                wih_view = wih_d.ap().rearrange("(k p) n -> p k n", p=128)
                encTs = []
                for sh, src_d in enumerate([(encT_first_d), (encT_own_d)]):
                    encT = pa.tile([128, 8, TS * B], BF16, tag=f"encT{sh}",
                                   name=f"encT{sh}")
                    nc.sync.dma_start(
                        out=encT[:],
                        in_=src_d.ap().rearrange("(k p) t b -> p k (t b)", p=128))
                    encTs.append(encT)
                for n in range(8):
                    wih_n = pa.tile([128, 8, 512], BF16, tag="wih_n",
                                    name="wih_n", bufs=2)
                    nc.sync.dma_start(out=wih_n[:],
                                      in_=wih_view[:, :, n * 512:(n + 1) * 512])
                    for sh, dst in enumerate([xp_first, xp_own]):
                        for m in range(8):
                            w4 = (n * 16 + sh * 8 + m) % 4
                            sl = ps[:, w4 * 512:(w4 + 1) * 512]
                            for k in range(8):
                                nc.tensor.matmul(
                                    sl, encTs[sh][:, k, m * 128:(m + 1) * 128],
                                    wih_n[:, k, :],
                                    start=(k == 0), stop=(k == 7))
                            xp_sb = pa.tile([128, 512], F32, tag="xp_sb",
                                            name="xp_sb", bufs=3)
                            nc.vector.tensor_add(
                                xp_sb[:], sl, bias[:, n * 512:(n + 1) * 512])
                            nc.sync.dma_start(
                                out=dst[m * 128:(m + 1) * 128, n * 512:(n + 1) * 512],
                                in_=xp_sb[:])
                nc.gpsimd.collective_compute(
                    "AllGather", mybir.AluOpType.bypass,
                    replica_groups=[list(range(N_CORES))],
                    ins=[xp_own.opt()], outs=[xp_ag.opt()])

            # ---------------- phase B: recurrence + FC head -------------------
            with tc.tile_pool(name="phb", bufs=1) as pb:
                for t in range(T):
                    n = int(n_t[t])
                    half = t % 2
                    if n > 0:
                        xp_src = xp_first if t < TS else xp_ag
                        xpt = pb.tile([128, 2048], F32, tag="xpt", name="xpt", bufs=2)
                        nc.sync.dma_start(
                            out=xpt[0:64, :], in_=xp_src[B * t:B * (t + 1), 0:2048])
                        nc.sync.dma_start(
                            out=xpt[64:128, :], in_=xp_src[B * t:B * (t + 1), 2048:4096])
                        for w in range(4):
                            sl = ps[:, w * 512:(w + 1) * 512]
                            for g in range(2):
                                for k in range(8):
                                    nc.tensor.matmul(
                                        sl[g * 64:(g + 1) * 64, :],
                                        hT[:, k, :],
                                        whh[:, k, g * 2048 + w * 512:
                                            g * 2048 + (w + 1) * 512],
                                        start=(k == 0), stop=(k == 7),
                                        tile_position=(0, g * 64))
                            nc.vector.tensor_add(
                                sl, sl, xpt[:, w * 512:(w + 1) * 512])
                        sig = pb.tile([128, 2048], F32, tag="sig", name="sig", bufs=1)
                        # cols 0:1024 = {f (top), i (bottom)} both sigmoid
                        nc.scalar.activation(sig[:, 0:1024], ps[:, 0:1024], SIG)
                        # cols 1024:2048 = {o (top) sigmoid, g (bottom) tanh}
                        nc.scalar.activation(sig[0:64, 1024:2048],
                                             ps[0:64, 1024:2048], SIG)
                        nc.scalar.activation(sig[64:128, 1024:2048],
                                             ps[64:128, 1024:2048], TANH)
                        fc_t = pb.tile([B, D], F32, tag="fc_t", name="fc_t", bufs=1)
                        ig = pb.tile([128, D], F32, tag="ig", name="ig", bufs=1)
                        igt = pb.tile([B, D], F32, tag="igt", name="igt", bufs=1)
                        th = pb.tile([B, D], F32, tag="th", name="th", bufs=1)
                        hbf = pb.tile([B, D], BF16, tag="hbf", name="hbf", bufs=1)
                        nc.vector.tensor_mul(fc_t[0:n, :], sig[0:n, 0:1024],
                                             cst[0:n, :])
                        nc.vector.tensor_mul(ig[64:64 + n, :], sig[64:64 + n, 0:1024],
                                             sig[64:64 + n, 1024:2048])
                        nc.sync.dma_start(out=igt[0:n, :], in_=ig[64:64 + n, :])
                        nc.vector.tensor_add(cst[0:n, :], fc_t[0:n, :], igt[0:n, :])
                        nc.scalar.activation(th[0:n, :], cst[0:n, :], TANH)
                        nc.vector.tensor_mul(hbf[0:n, :], sig[0:n, 1024:2048],
                                             th[0:n, :])
                        ps_t = psum.tile([128, 8, B], BF16, tag="ps_t",
                                         name="ps_t", bufs=1)
                        for k in range(8):
                            nc.tensor.transpose(
                                ps_t[:, k, 0:n], hbf[0:n, k * 128:(k + 1) * 128],
                                ident[0:n, 0:n])
                        nc.scalar.copy(hT[:, :, 0:n], ps_t[:, :, 0:n])
                    nc.vector.tensor_copy(hT2[:, :, half * B:(half + 1) * B], hT[:])
                    if half == 1:
                        fps = psum.tile([128, VC], F32, tag="fps", name="fps", bufs=1)
                        for nn in range(3):
                            c0, c1 = nn * 512, min((nn + 1) * 512, VC)
                            for k in range(8):
                                nc.tensor.matmul(
                                    fps[:, c0:c1], hT2[:, k, :], wfc[:, k, c0:c1],
                                    start=(k == 0), stop=(k == 7))
                        prd = pb.tile([128, VC], F32, tag="prd", name="prd", bufs=2)
                        nc.scalar.copy(prd[:], fps[:])
                        nc.sync.dma_start(out=out_d.ap()[:, t - 1, :],
                                          in_=prd[0:64, :])
                        nc.sync.dma_start(out=out_d.ap()[:, t, :],
                                          in_=prd[64:128, :])

    nc.compile()
    return nc


def _host_prep(inputs):
    enc = np.asarray(inputs["encoder_out"], np.float32)
    lens = np.asarray(inputs["caption_lengths"]).reshape(-1).astype(np.int64)
    W_ih = np.asarray(inputs["W_ih"], np.float32)
    b_ih = np.asarray(inputs["b_ih"], np.float32)
    W_hh = np.asarray(inputs["W_hh"], np.float32)
    b_hh = np.asarray(inputs["b_hh"], np.float32)
    W_fc = np.asarray(inputs["W_fc"], np.float32)
    b_fc = np.asarray(inputs["b_fc"], np.float32)

    sort_ind = np.argsort(-lens, kind="stable")
    dec_len = lens[sort_ind] - 1
    n_t = np.array([(dec_len > t).sum() for t in range(T)], np.int64)
    enc_s = enc[sort_ind]

    # gate permutation i,f,g,o -> f,o,i,g
    perm = np.concatenate([np.arange(D, 2 * D), np.arange(3 * D, 4 * D),
                           np.arange(0, D), np.arange(2 * D, 3 * D)])
    wihT = np.ascontiguousarray(W_ih[perm].T).astype(BF_NP)          # [E, 4D]
    whhT = np.ascontiguousarray(W_hh[perm].T).astype(BF_NP)          # [D, 4D]
    bias = (b_ih + b_hh)[perm].astype(np.float32)
    bias_rep = np.ascontiguousarray(np.broadcast_to(bias, (128, G4)))

    wfc_pad = np.zeros((VP, D), np.float32)
    wfc_pad[:V] = W_fc
    bfc_pad = np.zeros((VP,), np.float32)
    bfc_pad[:V] = b_fc

    encT = np.ascontiguousarray(enc_s.transpose(2, 1, 0)).astype(BF_NP)  # [E,T,B]
    ident = np.eye(B, dtype=BF_NP)

    in_maps = []
    for c in range(N_CORES):
        wfcT_c = np.ascontiguousarray(
            wfc_pad[c * VC:(c + 1) * VC].T).astype(BF_NP)            # [D, VC]
        in_maps.append({
            "encT_own": np.ascontiguousarray(encT[:, c * TS:(c + 1) * TS, :]),
            "encT_first": np.ascontiguousarray(encT[:, 0:TS, :]),
            "wihT": wihT,
            "whhT": whhT,
            "wfcT": wfcT_c,
            "bias_rep": bias_rep,
            "ident": ident,
        })
    return in_maps, n_t, bfc_pad


def kernel(**inputs):
    in_maps, n_t, bfc_pad = _host_prep(inputs)
    key = tuple(n_t.tolist())
    if key not in _CACHE:
        _CACHE[key] = _build(n_t)
    nc = _CACHE[key]
    res = run_bass_kernel_spmd(nc, in_maps, core_ids=list(range(N_CORES)))
    out = np.concatenate([res.results[c]["preds"] for c in range(N_CORES)],
                         axis=2)[:, :, :V]
    out = out.astype(np.float32)
    if bfc_pad.any():
        out += bfc_pad[:V]
    return np.ascontiguousarray(out)
